# revision 27
# baseline (speedup 1.0000x reference)
"""Trainium2 Bass kernel for nn_CurvatureLoss: softmax over 4 classes ->
3 probability maps -> fused curvature-stencil chain -> masked-mean loss.

Data-parallel over batch (8 samples -> 8 cores). Per core, 9 overlapping
128-row slabs. H-direction stencils (lap band, gy, -hyy) are fp16 band
matmuls on PE (kept dense so it ramps to 2.4 GHz). W-direction stencils
and products run on DVE with shifted fp16 APs (2x mode); three-term sums
are sign-folded into Pool tensor_sub ops (cheaper than Pool add). All
activation work (exp, copies, squares, ln, exp) stays in the single
`natural_log_exp_and_others` table set -> no ACT table reloads.
D^-1.5 is computed as exp(-1.5*ln(D)) with the +1 folded into Ln's bias.
Per-(slab,map) masked sums s = sum relu(-curv) and counts c ride free on
tensor_scalar accum_out columns; the host does the tiny masked-mean
reduction.
"""
import sys

if "/opt/trn_rl_repo" not in sys.path:
    sys.path.insert(0, "/opt/trn_rl_repo")

import numpy as np

P = 128
H = W = 1024
N_CORES = 8
STARTS = [0, 122, 244, 366, 488, 610, 732, 854, 896]
NSLAB = len(STARTS)
ACC_COLS = NSLAB * 3 * 2
SQ = 0.7071067811865476


def _band_weights():
    """fp16 lhsT weights [128, 4*128]: M1.T (lap band), I, M2.T (gy),
    M3n.T (-hyy band)."""
    SyP = np.eye(P, k=1, dtype=np.float64)   # (S+ x)[h] = x[h+1]
    SyM = np.eye(P, k=-1, dtype=np.float64)  # (S- x)[h] = x[h-1]
    I = np.eye(P, dtype=np.float64)
    M1 = SyP + SyM - 4 * I                   # lap = M1 @ p + (E + W)
    M2 = SyP - SyM                           # gy = M2 @ lap
    M3n = -((2 * I - SyP - SyM) @ M2)        # -hyy = M3n @ lap
    wts = np.concatenate([M1.T, I, M2.T, M3n.T], axis=1).astype(np.float16)
    return np.ascontiguousarray(wts)


def _row_masks():
    """[128, 3] fp32: -1 on owned rows (columns: first/middle/last slab)."""
    masks = np.zeros((P, 3), np.float32)
    masks[0:125, 0] = -1.0
    masks[3:125, 1] = -1.0
    masks[83:128, 2] = -1.0
    return masks


_CACHE = {}


def _build_program():
    import concourse.bacc as bacc
    import concourse.mybir as mybir
    from concourse.tile import TileContext
    from concourse.alu_op_type import AluOpType as Alu

    f32 = mybir.dt.float32
    f16 = mybir.dt.float16
    Act = mybir.ActivationFunctionType

    nc = bacc.Bacc("TRN2", target_bir_lowering=False, debug=False,
                   enable_asserts=False, num_devices=N_CORES)
    pred = nc.dram_tensor("pred", [4, H, W], f32, kind="ExternalInput").ap()
    wtsd = nc.dram_tensor("wts", [P, 4 * P], f16, kind="ExternalInput").ap()
    mskd = nc.dram_tensor("msk", [P, 3], f32, kind="ExternalInput").ap()
    accd = nc.dram_tensor("acc", [P, ACC_COLS], f32, kind="ExternalOutput").ap()

    W2 = W + 2

    with TileContext(nc) as tc:
        with tc.tile_pool(name="const", bufs=1) as cpool, \
             tc.tile_pool(name="work", bufs=2) as pool, \
             tc.tile_pool(name="front", bufs=2) as fpool, \
             tc.tile_pool(name="plap", bufs=1, space="PSUM") as plap, \
             tc.tile_pool(name="pgy", bufs=1, space="PSUM") as pgy, \
             tc.tile_pool(name="psmx", bufs=1, space="PSUM") as psum_pool, \
             tc.tile_pool(name="phy", bufs=1, space="PSUM") as phy, \
             nc.allow_low_precision(reason="fp16 chain validated vs reference"):
            _li = mybir.InstLoadActFuncSet(act_func_set_id=6)
            _li.engine = mybir.EngineType.Activation
            nc.scalar.add_instruction(_li)
            wt = cpool.tile([P, 4 * P], f16)
            nc.sync.dma_start(out=wt[:], in_=wtsd)
            wM1 = wt[:, 0:P]
            wI = wt[:, P:2 * P]
            wM2 = wt[:, 2 * P:3 * P]
            wM3n = wt[:, 3 * P:4 * P]
            mtile = cpool.tile([P, 3], f32)
            nc.sync.dma_start(out=mtile[:], in_=mskd)
            acc = cpool.tile([P, ACC_COLS], f32)
            nc.vector.memset(acc[:], 0.0)
            scr = cpool.tile([P, W], f16)   # relu scratch
            scr2 = cpool.tile([P, W], f16)  # dead output of accum-only TS ops
            hs = cpool.tile([P, 1], f32)    # sqrt(1/2) bias for squares
            nc.vector.memset(hs[:], SQ)
            one = cpool.tile([P, 1], f32)   # +1 bias for ln(D)
            nc.vector.memset(one[:], 1.0)

            # parity-buffered padded tiles; pads zeroed once here
            probs2 = [cpool.tile([P, 3, W2], f16, name=f"probs{i}")
                      for i in range(2)]
            lap2 = [cpool.tile([P, 3, W2], f16, name=f"lap{i}")
                    for i in range(2)]
            gx2 = [cpool.tile([P, 3, W2], f16, name=f"gx{i}")
                   for i in range(2)]
            for t in probs2 + lap2 + gx2:
                nc.gpsimd.memset(t[:, :, 0:1], 0.0)
                nc.gpsimd.memset(t[:, :, W + 1:W + 2], 0.0)

            def emit_front_softmax(si, st):
                probs = probs2[si % 2]
                lap3 = lap2[si % 2]
                gx3 = gx2[si % 2]
                xt = fpool.tile([P, 4, W], f32, tag="xt")
                for c in range(4):
                    nc.sync.dma_start(out=xt[:, c, :], in_=pred[c, st:st + P, :])
                ex = fpool.tile([P, 4, W], f16, tag="ex")
                nc.scalar.activation(out=ex[:], in_=xt[:], func=Act.Exp)
                sum_ps = psum_pool.tile([P, W], f32, tag="sumps")
                for hf in range(2):
                    sl = slice(hf * 512, (hf + 1) * 512)
                    for c in range(4):
                        nc.tensor.matmul(sum_ps[:, sl], lhsT=wI,
                                         rhs=ex[:, c, sl],
                                         start=(c == 0), stop=(c == 3))
                t03 = fpool.tile([P, W], f16, tag="t03")
                nc.scalar.activation(out=t03, in_=sum_ps[:], func=Act.Copy)
                nc.vector.reciprocal(t03, t03)               # r = 1/sum
                nc.gpsimd.tensor_mul(probs[:, 0, 1:W + 1], ex[:, 1, :], t03)
                nc.gpsimd.tensor_mul(probs[:, 1, 1:W + 1], ex[:, 2, :], t03)
                nc.gpsimd.tensor_mul(probs[:, 2, 1:W + 1], ex[:, 3, :], t03)

                A3 = pool.tile([P, 3, W], f16, tag="A3")
                C23 = pool.tile([P, 3, W], f16, tag="C23")
                SG23 = pool.tile([P, 3, W], f16, tag="SG23")
                m2_3 = pool.tile([P, 3, W], f16, tag="m2_3")
                m4_3 = pool.tile([P, 3, W], f16, tag="m4_3")
                return (A3, C23, SG23, m2_3, m4_3)

            def emit_map_window(si, m, tiles):
                A3, C23, SG23, m2_3, m4_3 = tiles
                probs = probs2[si % 2]
                lap3 = lap2[si % 2]
                gx3 = gx2[si % 2]
                if True:
                    srcs = [m] if m != 1 else [0, 1]
                    lap_ps = plap.tile([P, W], f32, tag="lapps")
                    seq = ([(wM1, sm, 1) for sm in srcs]
                           + [(wI, sm, 2) for sm in srcs]
                           + [(wI, sm, 0) for sm in srcs])
                    nmm = len(seq)
                    for hf in range(2):
                        sl = slice(hf * 512, (hf + 1) * 512)
                        for k, (wgt, sm, off) in enumerate(seq):
                            rhs = probs[:, sm, off:off + W]
                            nc.tensor.matmul(lap_ps[:, sl], lhsT=wgt,
                                             rhs=rhs[:, sl],
                                             start=(k == 0),
                                             stop=(k == nmm - 1))
                    lC = lap3[:, m, 1:W + 1]
                    nc.scalar.activation(out=lC, in_=lap_ps[:], func=Act.Copy)
                    gxm = gx3[:, m, 1:W + 1]
                    nc.vector.tensor_sub(gxm, lap3[:, m, 2:W + 2],
                                         lap3[:, m, 0:W])
                    nc.scalar.activation(out=C23[:, m, :], in_=gxm,
                                         func=Act.Square, scale=SQ, bias=hs[:])
                    gy_ps = pgy.tile([P, W], f32, tag="gyps")
                    hy_ps = phy.tile([P, W], f32, tag="hyps")
                    for hf in range(2):
                        sl = slice(hf * 512, (hf + 1) * 512)
                        nc.tensor.matmul(gy_ps[:, sl], lhsT=wM2, rhs=lC[:, sl],
                                         start=True, stop=True)
                    for hf in range(2):
                        sl = slice(hf * 512, (hf + 1) * 512)
                        nc.tensor.matmul(hy_ps[:, sl], lhsT=wM3n, rhs=lC[:, sl],
                                         start=True, stop=True)
                    nc.scalar.activation(out=A3[:, m, :], in_=gy_ps[:],
                                         func=Act.Square, scale=SQ, bias=hs[:])
                    nc.scalar.activation(out=SG23[:, m, :], in_=gy_ps[:],
                                         func=Act.Square)
                    nc.vector.tensor_mul(m2_3[:, m, :], gxm, gy_ps[:])
                    nc.vector.tensor_mul(m4_3[:, m, :], hy_ps[:], C23[:, m, :])

            def tail_chunk0(si, tiles):
                A3, C23, SG23, m2_3, m4_3 = tiles
                gx3 = gx2[si % 2]
                gC = gx3[:, :, 1:W + 1]
                gE = gx3[:, :, 2:W + 2]
                gW_ = gx3[:, :, 0:W]
                hxy = pool.tile([P, 3, W], f16, tag="hxy")
                nc.gpsimd.tensor_sub(hxy, gE, gW_)
                Sb = pool.tile([P, 3, W], f16, tag="Sb")
                nc.vector.tensor_add(Sb, gE, gW_)
                d3 = pool.tile([P, 3, W], f16, tag="d3")
                nc.vector.tensor_scalar(out=d3, in0=gC, scalar1=2.0,
                                        scalar2=None, op0=Alu.mult)
                nc.gpsimd.tensor_sub(d3, d3, Sb)             # hxx
                return hxy, d3

            def tail_chunk1(si, tiles, scratch):
                A3, C23, SG23, m2_3, m4_3 = tiles
                hxy, d3 = scratch
                gx3 = gx2[si % 2]
                gC = gx3[:, :, 1:W + 1]
                SGx = pool.tile([P, 3, W], f16, tag="SGx")
                for m in range(3):
                    nc.scalar.activation(out=SGx[:, m, :],
                                         in_=gx3[:, m, 1:W + 1],
                                         func=Act.Square)
                nc.vector.tensor_add(SG23, SGx, SG23)        # Dp = gx^2+gy^2
                nc.scalar.activation(out=C23, in_=SG23, func=Act.Ln,
                                     bias=one[:])            # ln(D)
                nc.vector.tensor_mul(hxy, m2_3, hxy)         # m3

            def tail_chunk2(si, tiles, scratch):
                A3, C23, SG23, m2_3, m4_3 = tiles
                hxy, d3 = scratch
                mk = mtile[:, (0 if si == 0 else (2 if si == NSLAB - 1 else 1))
                           ][:, None]
                nc.scalar.activation(out=SG23, in_=C23, func=Act.Exp,
                                     scale=-1.5)             # D^-1.5
                nc.vector.tensor_mul(A3, d3, A3)             # m1
                nc.vector.tensor_sub(A3, A3, hxy)            # a1 = m1 - m3
                nc.vector.tensor_sub(A3, A3, m4_3)           # a2 = num/2
                nc.vector.tensor_mul(A3, A3, SG23)           # y = curv
                for m in range(3):
                    col = (si * 3 + m) * 2
                    # n = relu(mk*y) (plain 2-op TS, no accum)
                    nc.vector.tensor_scalar(
                        out=scr, in0=A3[:, m, :], scalar1=mk, scalar2=0.0,
                        op0=Alu.mult, op1=Alu.max)
                    # s = sum(n): op1 is the reduce op when accum_out is set
                    nc.vector.tensor_scalar(
                        out=scr2, in0=scr, scalar1=1.0, scalar2=None,
                        op0=Alu.mult, op1=Alu.add,
                        accum_out=acc[:, col:col + 1])
                    # count = sum(n > 0)
                    nc.vector.tensor_scalar(
                        out=scr2, in0=scr, scalar1=0.0, scalar2=None,
                        op0=Alu.is_gt, op1=Alu.add,
                        accum_out=acc[:, col + 1:col + 2])

            pending = None
            for si, st in enumerate(STARTS):
                tiles = emit_front_softmax(si, st)
                for m in range(3):
                    emit_map_window(si, m, tiles)
                    if pending is not None:
                        ptiles, pscr = pending
                        if m == 0:
                            pscr2 = tail_chunk0(si - 1, ptiles)
                            pending = (ptiles, pscr2)
                        elif m == 1:
                            tail_chunk1(si - 1, ptiles, pending[1])
                        else:
                            tail_chunk2(si - 1, ptiles, pending[1])
                pending = (tiles, None)
            ptiles, _ = pending
            s2 = tail_chunk0(NSLAB - 1, ptiles)
            tail_chunk1(NSLAB - 1, ptiles, s2)
            tail_chunk2(NSLAB - 1, ptiles, s2)

            nc.sync.dma_start(out=accd, in_=acc[:])
    nc.compile()
    return nc


def _get_program():
    if "nc" not in _CACHE:
        _CACHE["nc"] = _build_program()
    return _CACHE["nc"]


def _run_device(pred_np):
    from concourse import bass_utils
    nc = _get_program()
    wts = _band_weights()
    msk = _row_masks()
    in_maps = [{"pred": np.ascontiguousarray(pred_np[b]), "wts": wts,
                "msk": msk}
               for b in range(N_CORES)]
    res = bass_utils.run_bass_kernel_spmd(nc, in_maps,
                                          core_ids=list(range(N_CORES)))
    return [res.results[b]["acc"] for b in range(N_CORES)]


def _host_reduce(accs):
    total = 0.0
    for b in range(N_CORES):
        a = accs[b].astype(np.float64)
        for m in range(3):
            s = a[:, [(si * 3 + m) * 2 for si in range(NSLAB)]].sum()
            c = a[:, [(si * 3 + m) * 2 + 1 for si in range(NSLAB)]].sum()
            if c > 0:
                total += s / max(c, 1.0)
    return np.float32(total)


def kernel(pred, target=None):
    assert pred.shape == (N_CORES, 4, H, W)
    accs = _run_device(np.asarray(pred, dtype=np.float32))
    return _host_reduce(accs)


# revision 30
# speedup vs baseline: 1.0434x; 1.0434x over previous
"""Trainium2 Bass kernel for nn_CurvatureLoss: softmax over 4 classes ->
3 probability maps -> fused curvature-stencil chain -> masked-mean loss.

Data-parallel over batch (8 samples -> 8 cores). Per core, 9 overlapping
128-row slabs. H-direction stencils (lap band, gy, -hyy) are fp16 band
matmuls on PE (kept dense so it ramps to 2.4 GHz). W-direction stencils
and products run on DVE with shifted fp16 APs (2x mode); three-term sums
are sign-folded into Pool tensor_sub ops (cheaper than Pool add). All
activation work (exp, copies, squares, ln, exp) stays in the single
`natural_log_exp_and_others` table set -> no ACT table reloads.
D^-1.5 is computed as exp(-1.5*ln(D)) with the +1 folded into Ln's bias.
Per-(slab,map) masked sums s = sum relu(-curv) and counts c ride free on
tensor_scalar accum_out columns; the host does the tiny masked-mean
reduction.
"""
import sys

if "/opt/trn_rl_repo" not in sys.path:
    sys.path.insert(0, "/opt/trn_rl_repo")

import numpy as np

P = 128
H = W = 1024
N_CORES = 8
STARTS = [0, 122, 244, 366, 488, 610, 732, 854, 896]
NSLAB = len(STARTS)
ACC_COLS = NSLAB * 3 * 2
SQ = 0.7071067811865476


def _band_weights():
    """fp16 lhsT weights [128, 4*128]: M1.T (lap band), I, M2.T (gy),
    M3n.T (-hyy band)."""
    SyP = np.eye(P, k=1, dtype=np.float64)   # (S+ x)[h] = x[h+1]
    SyM = np.eye(P, k=-1, dtype=np.float64)  # (S- x)[h] = x[h-1]
    I = np.eye(P, dtype=np.float64)
    M1 = SyP + SyM - 4 * I                   # lap = M1 @ p + (E + W)
    M2 = SyP - SyM                           # gy = M2 @ lap
    M3n = -((2 * I - SyP - SyM) @ M2)        # -hyy = M3n @ lap
    wts = np.concatenate([M1.T, I, M2.T, M3n.T], axis=1).astype(np.float16)
    return np.ascontiguousarray(wts)


def _row_masks():
    """[128, 3] fp32: -1 on owned rows (columns: first/middle/last slab)."""
    masks = np.zeros((P, 3), np.float32)
    masks[0:125, 0] = -1.0
    masks[3:125, 1] = -1.0
    masks[83:128, 2] = -1.0
    return masks


_CACHE = {}


def _build_program():
    import concourse.bacc as bacc
    import concourse.mybir as mybir
    from concourse.tile import TileContext
    from concourse.alu_op_type import AluOpType as Alu

    f32 = mybir.dt.float32
    f16 = mybir.dt.float16
    Act = mybir.ActivationFunctionType

    nc = bacc.Bacc("TRN2", target_bir_lowering=False, debug=False,
                   enable_asserts=False, num_devices=N_CORES)
    pred = nc.dram_tensor("pred", [4, H, W], f32, kind="ExternalInput").ap()
    wtsd = nc.dram_tensor("wts", [P, 4 * P], f16, kind="ExternalInput").ap()
    mskd = nc.dram_tensor("msk", [P, 3], f32, kind="ExternalInput").ap()
    accd = nc.dram_tensor("acc", [P, ACC_COLS], f32, kind="ExternalOutput").ap()

    W2 = W + 2

    with TileContext(nc) as tc:
        with tc.tile_pool(name="const", bufs=1) as cpool, \
             tc.tile_pool(name="work", bufs=2) as pool, \
             tc.tile_pool(name="front", bufs=2) as fpool, \
             tc.tile_pool(name="plap", bufs=1, space="PSUM") as plap, \
             tc.tile_pool(name="pgy", bufs=1, space="PSUM") as pgy, \
             tc.tile_pool(name="psmx", bufs=1, space="PSUM") as psum_pool, \
             tc.tile_pool(name="phy", bufs=1, space="PSUM") as phy, \
             nc.allow_low_precision(reason="fp16 chain validated vs reference"):
            _li = mybir.InstLoadActFuncSet(act_func_set_id=6)
            _li.engine = mybir.EngineType.Activation
            nc.scalar.add_instruction(_li)
            wt = cpool.tile([P, 4 * P], f16)
            nc.sync.dma_start(out=wt[:], in_=wtsd)
            wM1 = wt[:, 0:P]
            wI = wt[:, P:2 * P]
            wM2 = wt[:, 2 * P:3 * P]
            wM3n = wt[:, 3 * P:4 * P]
            mtile = cpool.tile([P, 3], f32)
            nc.sync.dma_start(out=mtile[:], in_=mskd)
            acc = cpool.tile([P, ACC_COLS], f32)
            nc.vector.memset(acc[:], 0.0)
            scr = cpool.tile([P, W], f16)   # relu scratch
            scr2 = cpool.tile([P, W], f16)  # dead output of accum-only TS ops
            hs = cpool.tile([P, 1], f32)    # sqrt(1/2) bias for squares
            nc.vector.memset(hs[:], SQ)
            one = cpool.tile([P, 1], f32)   # +1 bias for ln(D)
            nc.vector.memset(one[:], 1.0)

            # parity-buffered padded tiles; pads zeroed once here
            probs2 = [cpool.tile([P, 3, W2], f16, name=f"probs{i}")
                      for i in range(2)]
            lap2 = [cpool.tile([P, 3, W2], f16, name=f"lap{i}")
                    for i in range(2)]
            gx2 = [cpool.tile([P, 3, W2], f16, name=f"gx{i}")
                   for i in range(2)]
            for t in probs2 + lap2 + gx2:
                nc.gpsimd.memset(t[:, :, 0:1], 0.0)
                nc.gpsimd.memset(t[:, :, W + 1:W + 2], 0.0)

            def emit_front_softmax(si, st):
                probs = probs2[si % 2]
                lap3 = lap2[si % 2]
                gx3 = gx2[si % 2]
                xt = fpool.tile([P, 4, W], f32, tag="xt")
                for c in range(4):
                    nc.sync.dma_start(out=xt[:, c, :], in_=pred[c, st:st + P, :])
                ex = fpool.tile([P, 4, W], f16, tag="ex")
                nc.scalar.activation(out=ex[:], in_=xt[:], func=Act.Exp)
                sum_ps = psum_pool.tile([P, W], f32, tag="sumps")
                for hf in range(2):
                    sl = slice(hf * 512, (hf + 1) * 512)
                    for c in range(4):
                        nc.tensor.matmul(sum_ps[:, sl], lhsT=wI,
                                         rhs=ex[:, c, sl],
                                         start=(c == 0), stop=(c == 3))
                t03 = fpool.tile([P, W], f16, tag="t03")
                nc.vector.reciprocal(t03, sum_ps[:])         # r = 1/sum
                nc.gpsimd.tensor_mul(probs[:, 0, 1:W + 1], ex[:, 1, :], t03)
                nc.gpsimd.tensor_mul(probs[:, 1, 1:W + 1], ex[:, 2, :], t03)
                nc.gpsimd.tensor_mul(probs[:, 2, 1:W + 1], ex[:, 3, :], t03)

                A3 = pool.tile([P, 3, W], f16, tag="A3")
                C23 = pool.tile([P, 3, W], f16, tag="C23")
                SG23 = pool.tile([P, 3, W], f16, tag="SG23")
                m2_3 = pool.tile([P, 3, W], f16, tag="m2_3")
                m4_3 = pool.tile([P, 3, W], f16, tag="m4_3")
                return (A3, C23, SG23, m2_3, m4_3)

            def emit_map_window(si, m, tiles):
                A3, C23, SG23, m2_3, m4_3 = tiles
                probs = probs2[si % 2]
                lap3 = lap2[si % 2]
                gx3 = gx2[si % 2]
                if True:
                    srcs = [m] if m != 1 else [0, 1]
                    lap_ps = plap.tile([P, W], f32, tag="lapps")
                    seq = ([(wM1, sm, 1) for sm in srcs]
                           + [(wI, sm, 2) for sm in srcs]
                           + [(wI, sm, 0) for sm in srcs])
                    nmm = len(seq)
                    for hf in range(2):
                        sl = slice(hf * 512, (hf + 1) * 512)
                        for k, (wgt, sm, off) in enumerate(seq):
                            rhs = probs[:, sm, off:off + W]
                            nc.tensor.matmul(lap_ps[:, sl], lhsT=wgt,
                                             rhs=rhs[:, sl],
                                             start=(k == 0),
                                             stop=(k == nmm - 1))
                    lC = lap3[:, m, 1:W + 1]
                    nc.scalar.activation(out=lC, in_=lap_ps[:], func=Act.Copy)
                    gxm = gx3[:, m, 1:W + 1]
                    nc.vector.tensor_sub(gxm, lap3[:, m, 2:W + 2],
                                         lap3[:, m, 0:W])
                    nc.scalar.activation(out=C23[:, m, :], in_=gxm,
                                         func=Act.Square, scale=SQ, bias=hs[:])
                    gy_ps = pgy.tile([P, W], f32, tag="gyps")
                    hy_ps = phy.tile([P, W], f32, tag="hyps")
                    for hf in range(2):
                        sl = slice(hf * 512, (hf + 1) * 512)
                        nc.tensor.matmul(gy_ps[:, sl], lhsT=wM2, rhs=lC[:, sl],
                                         start=True, stop=True)
                    for hf in range(2):
                        sl = slice(hf * 512, (hf + 1) * 512)
                        nc.tensor.matmul(hy_ps[:, sl], lhsT=wM3n, rhs=lC[:, sl],
                                         start=True, stop=True)
                    nc.scalar.activation(out=A3[:, m, :], in_=gy_ps[:],
                                         func=Act.Square, scale=SQ, bias=hs[:])
                    nc.scalar.activation(out=SG23[:, m, :], in_=gy_ps[:],
                                         func=Act.Square)
                    nc.vector.tensor_mul(m2_3[:, m, :], gxm, gy_ps[:])
                    nc.vector.tensor_mul(m4_3[:, m, :], hy_ps[:], C23[:, m, :])

            def tail_chunk0(si, tiles):
                A3, C23, SG23, m2_3, m4_3 = tiles
                gx3 = gx2[si % 2]
                gC = gx3[:, :, 1:W + 1]
                gE = gx3[:, :, 2:W + 2]
                gW_ = gx3[:, :, 0:W]
                hxy = pool.tile([P, 3, W], f16, tag="hxy")
                nc.gpsimd.tensor_sub(hxy, gE, gW_)
                Sb = pool.tile([P, 3, W], f16, tag="Sb")
                nc.vector.tensor_add(Sb, gE, gW_)
                d3 = pool.tile([P, 3, W], f16, tag="d3")
                nc.vector.tensor_scalar(out=d3, in0=gC, scalar1=2.0,
                                        scalar2=None, op0=Alu.mult)
                nc.gpsimd.tensor_sub(d3, d3, Sb)             # hxx
                return hxy, d3

            def tail_chunk1(si, tiles, scratch):
                A3, C23, SG23, m2_3, m4_3 = tiles
                hxy, d3 = scratch
                gx3 = gx2[si % 2]
                gC = gx3[:, :, 1:W + 1]
                SGx = pool.tile([P, 3, W], f16, tag="SGx")
                for m in range(3):
                    nc.scalar.activation(out=SGx[:, m, :],
                                         in_=gx3[:, m, 1:W + 1],
                                         func=Act.Square)
                nc.vector.tensor_add(SG23, SGx, SG23)        # Dp = gx^2+gy^2
                nc.scalar.activation(out=C23, in_=SG23, func=Act.Ln,
                                     bias=one[:])            # ln(D)
                nc.vector.tensor_mul(hxy, m2_3, hxy)         # m3

            def tail_chunk2(si, tiles, scratch):
                A3, C23, SG23, m2_3, m4_3 = tiles
                hxy, d3 = scratch
                mk = mtile[:, (0 if si == 0 else (2 if si == NSLAB - 1 else 1))
                           ][:, None]
                nc.scalar.activation(out=SG23, in_=C23, func=Act.Exp,
                                     scale=-1.5)             # D^-1.5
                nc.vector.tensor_mul(A3, d3, A3)             # m1
                nc.vector.tensor_sub(A3, A3, hxy)            # a1 = m1 - m3
                nc.vector.tensor_sub(A3, A3, m4_3)           # a2 = num/2
                nc.vector.tensor_mul(A3, A3, SG23)           # y = curv
                for m in range(3):
                    col = (si * 3 + m) * 2
                    # n = relu(mk*y) (plain 2-op TS, no accum)
                    nc.vector.tensor_scalar(
                        out=scr, in0=A3[:, m, :], scalar1=mk, scalar2=0.0,
                        op0=Alu.mult, op1=Alu.max)
                    # s = sum(n): op1 is the reduce op when accum_out is set
                    nc.vector.tensor_scalar(
                        out=scr2, in0=scr, scalar1=1.0, scalar2=None,
                        op0=Alu.mult, op1=Alu.add,
                        accum_out=acc[:, col:col + 1])
                    # count = sum(n > 0)
                    nc.vector.tensor_scalar(
                        out=scr2, in0=scr, scalar1=0.0, scalar2=None,
                        op0=Alu.is_gt, op1=Alu.add,
                        accum_out=acc[:, col + 1:col + 2])

            pending = None
            for si, st in enumerate(STARTS):
                tiles = emit_front_softmax(si, st)
                for m in range(3):
                    emit_map_window(si, m, tiles)
                    if pending is not None:
                        ptiles, pscr = pending
                        if m == 0:
                            pscr2 = tail_chunk0(si - 1, ptiles)
                            pending = (ptiles, pscr2)
                        elif m == 1:
                            tail_chunk1(si - 1, ptiles, pending[1])
                        else:
                            tail_chunk2(si - 1, ptiles, pending[1])
                pending = (tiles, None)
            ptiles, _ = pending
            s2 = tail_chunk0(NSLAB - 1, ptiles)
            tail_chunk1(NSLAB - 1, ptiles, s2)
            tail_chunk2(NSLAB - 1, ptiles, s2)

            nc.sync.dma_start(out=accd, in_=acc[:])
    nc.compile()
    return nc


def _get_program():
    if "nc" not in _CACHE:
        _CACHE["nc"] = _build_program()
    return _CACHE["nc"]


def _run_device(pred_np):
    from concourse import bass_utils
    nc = _get_program()
    wts = _band_weights()
    msk = _row_masks()
    in_maps = [{"pred": np.ascontiguousarray(pred_np[b]), "wts": wts,
                "msk": msk}
               for b in range(N_CORES)]
    res = bass_utils.run_bass_kernel_spmd(nc, in_maps,
                                          core_ids=list(range(N_CORES)))
    return [res.results[b]["acc"] for b in range(N_CORES)]


def _host_reduce(accs):
    total = 0.0
    for b in range(N_CORES):
        a = accs[b].astype(np.float64)
        for m in range(3):
            s = a[:, [(si * 3 + m) * 2 for si in range(NSLAB)]].sum()
            c = a[:, [(si * 3 + m) * 2 + 1 for si in range(NSLAB)]].sum()
            if c > 0:
                total += s / max(c, 1.0)
    return np.float32(total)


def kernel(pred, target=None):
    assert pred.shape == (N_CORES, 4, H, W)
    accs = _run_device(np.asarray(pred, dtype=np.float32))
    return _host_reduce(accs)
